# revision 8
# baseline (speedup 1.0000x reference)
"""BEVLiftNet Trainium2 kernel, v2.

Sharding: 8 cores = 2 batches x 4 channel-groups (16 ch each).
Per core: lift all 4 cameras of its batch (depths -> voxel linear ids),
then scatter-add 16-channel bf16 feature rows into R replica DRAM grids
[Z*X*Y, 16] via single CCE-add indirect DMAs (the DMA engine does the
read-modify-write inline, so each chunk costs ONE SWDGE call instead of
the gather+add+scatter round trip).  Within-chunk duplicate rows are
pre-merged with a selection-matrix matmul; loser rows and invalid pixels
are clamped to an in-bounds trash row (never read back) so no DMA error
notifications fire.  Round-robin over R replica grids keeps chains independent so the
GpSimd SWDGE emission pipeline never stalls on DMA completion.  A final
pass sums the replicas, max-reduces over Z, and writes f32; the host
reassembles the [1, B*C, X, Y] output (flips + channel concat).
"""

import os
import sys

sys.path.insert(0, "/opt/trn_rl_repo")

import numpy as np
import ml_dtypes

import concourse.bacc as bacc
import concourse.bass as bass
import concourse.mybir as mybir
import concourse.tile as tile
from concourse import bass_utils
from concourse.masks import make_identity

B, N, C, H, W = 2, 4, 64, 128, 352
X, Y, Z = 256, 256, 8
CG = 16           # channels per core
NCAM = 4          # cameras per core (one batch)
HW = H * W        # 45056 pixels per camera
NTOK = NCAM * HW  # 180224 tokens per core
P = 128
FCAM = HW // P    # 352 chunk-columns per camera
F = NCAM * FCAM   # 1408 chunk-columns total
NROWS = Z * X * Y          # 524288 voxel rows per grid
SENT = 8_000_000           # OOB sentinel (> NROWS-1, f32-exact)
R = int(os.environ.get("KBEV_R", "3"))    # replica grids / chains
NF = int(os.environ.get("KBEV_NF", str(F)))   # chunks to scatter (debug)
NOMERGE = os.environ.get("KBEV_NOMERGE", "") == "1"
f32 = mybir.dt.float32
bf16 = mybir.dt.bfloat16
i32 = mybir.dt.int32
AL = mybir.AluOpType
ACT = mybir.ActivationFunctionType

_prog_cache = None


def _build_program():
    nc = bacc.Bacc("TRN2", target_bir_lowering=False, debug=False)
    feat = nc.dram_tensor("feat", [NTOK, CG], bf16, kind="ExternalInput")
    dep = nc.dram_tensor("dep", [NTOK], f32, kind="ExternalInput")
    kmat = nc.dram_tensor("kmat", [NCAM, 3, 3], f32, kind="ExternalInput")
    emat = nc.dram_tensor("emat", [NCAM, 4, 4], f32, kind="ExternalInput")
    tmat_t = nc.dram_tensor("tmat_t", [NCAM, 4, 4], f32, kind="ExternalInput")
    out = nc.dram_tensor("out", [X * Y, CG], f32, kind="ExternalOutput")
    grids = [nc.dram_tensor(f"grid{r}", [NROWS + 8, CG], bf16,
                            kind="ExternalInput") for r in range(R)]
    with tile.TileContext(nc) as tc:
        _emit(tc, feat.ap(), dep.ap(), kmat.ap(), emat.ap(), tmat_t.ap(),
              out.ap(), [g.ap() for g in grids])
    nc.compile()
    return nc


def _floor(nc, wp, out_t, in_t, n):
    """out = floor(in), robust to any f32->i32 rounding mode."""
    ii = wp.tile([P, n], i32, tag="fl_i")
    nc.vector.tensor_copy(ii[:], in_t)
    ff = wp.tile([P, n], f32, tag="fl_f")
    nc.vector.tensor_copy(ff[:], ii[:])
    gt = wp.tile([P, n], f32, tag="fl_g")
    nc.vector.tensor_tensor(out=gt[:], in0=ff[:], in1=in_t, op=AL.is_gt)
    nc.vector.tensor_tensor(out=out_t, in0=ff[:], in1=gt[:], op=AL.subtract)


def _lift_cam(tc, wp, dtile, cam_consts, lf, cam):
    """Compute one camera's voxel linear index (f32), invalid -> SENT."""
    nc = tc.nc
    if True:
        if True:
            cs = slice(cam * FCAM, (cam + 1) * FCAM)
            bc = cam_consts[cam]
            d = dtile[:, cs]
            idl = wp.tile([P, FCAM], i32, tag="idl")
            nc.gpsimd.iota(idl[:], pattern=[[1, FCAM]], base=0,
                           channel_multiplier=FCAM)
            idf = wp.tile([P, FCAM], f32, tag="idf")
            nc.scalar.copy(idf[:], idl[:])
            yf = wp.tile([P, FCAM], f32, tag="yf")
            xf = wp.tile([P, FCAM], f32, tag="xf")
            tmp = wp.tile([P, FCAM], f32, tag="t0")
            nc.scalar.activation(tmp[:], idf[:], ACT.Copy, bias=0.5 / W,
                                 scale=1.0 / W)
            _floor(nc, wp, yf[:], tmp[:], FCAM)
            nc.vector.scalar_tensor_tensor(out=xf[:], in0=yf[:],
                                           scalar=-float(W), in1=idf[:],
                                           op0=AL.mult, op1=AL.add)
            gs = []
            for i in range(3):
                a = wp.tile([P, FCAM], f32, tag="a_i")
                nc.vector.tensor_scalar(out=a[:], in0=xf[:],
                                        scalar1=bc[:, 3 * i:3 * i + 1],
                                        scalar2=None, op0=AL.mult)
                nc.vector.scalar_tensor_tensor(
                    out=a[:], in0=yf[:], scalar=bc[:, 3 * i + 1:3 * i + 2],
                    in1=a[:], op0=AL.mult, op1=AL.add)
                nc.vector.tensor_scalar(out=a[:], in0=a[:],
                                        scalar1=bc[:, 3 * i + 2:3 * i + 3],
                                        scalar2=None, op0=AL.add)
                e = wp.tile([P, FCAM], f32, tag="e_i")
                nc.vector.tensor_tensor(out=e[:], in0=a[:], in1=d, op=AL.mult)
                nc.vector.tensor_scalar(out=e[:], in0=e[:],
                                        scalar1=bc[:, 9 + i:10 + i],
                                        scalar2=None, op0=AL.add)
                mid = (X / 2.0, Y / 2.0, Z / 2.0)[i]
                g = wp.tile([P, FCAM], f32, tag=f"g_{i}")
                nc.scalar.activation(g[:], e[:], ACT.Copy, bias=mid, scale=2.0)
                gs.append(g)
            gx, gy, gz = gs
            v = wp.tile([P, FCAM], f32, tag="v")
            nc.vector.tensor_scalar(out=v[:], in0=gx[:], scalar1=-1.0,
                                    scalar2=None, op0=AL.is_gt)
            nc.vector.scalar_tensor_tensor(out=v[:], in0=gx[:],
                                           scalar=float(X), in1=v[:],
                                           op0=AL.is_lt, op1=AL.logical_and)
            for gg, bound in ((gy, float(Y)), (gz, float(Z))):
                v2 = wp.tile([P, FCAM], f32, tag="v2")
                nc.vector.tensor_scalar(out=v2[:], in0=gg[:], scalar1=-1.0,
                                        scalar2=None, op0=AL.is_gt)
                nc.vector.scalar_tensor_tensor(out=v2[:], in0=gg[:],
                                               scalar=bound, in1=v2[:],
                                               op0=AL.is_lt,
                                               op1=AL.logical_and)
                nc.vector.tensor_tensor(out=v[:], in0=v[:], in1=v2[:],
                                        op=AL.logical_and)
            fx = wp.tile([P, FCAM], f32, tag="fx")
            fy = wp.tile([P, FCAM], f32, tag="fy")
            fz = wp.tile([P, FCAM], f32, tag="fz")
            _floor(nc, wp, fx[:], gx[:], FCAM)
            _floor(nc, wp, fy[:], gy[:], FCAM)
            _floor(nc, wp, fz[:], gz[:], FCAM)
            for ft in (fx, fy, fz):
                nc.vector.tensor_scalar(out=ft[:], in0=ft[:], scalar1=0.0,
                                        scalar2=255.0, op0=AL.max, op1=AL.min)
            nc.vector.scalar_tensor_tensor(out=lf, in0=fz[:], scalar=float(X),
                                           in1=fx[:], op0=AL.mult, op1=AL.add)
            nc.vector.scalar_tensor_tensor(out=lf, in0=lf, scalar=float(Y),
                                           in1=fy[:], op0=AL.mult, op1=AL.add)
            # blend invalid -> SENT :  lin = SENT + v*(lin-SENT)
            nc.vector.tensor_scalar(out=lf, in0=lf, scalar1=-float(SENT),
                                    scalar2=None, op0=AL.add)
            nc.vector.tensor_tensor(out=lf, in0=lf, in1=v[:], op=AL.mult)
            nc.vector.tensor_scalar(out=lf, in0=lf, scalar1=float(SENT),
                                    scalar2=None, op0=AL.add)


def _emit(tc, feat, dep, kmat, emat, tmat_t, out, grids):
    nc = tc.nc

    with tc.tile_pool(name="persist", bufs=1) as pp, \
         tc.tile_pool(name="psc", bufs=2, space="PSUM") as psc:

        ident = pp.tile([P, P], f32, tag="ident")
        make_identity(nc, ident[:])
        iotl = pp.tile([P, P], i32, tag="iotl")
        nc.gpsimd.iota(iotl[:], pattern=[[1, P]], base=0, channel_multiplier=0)
        iota_f = pp.tile([P, P], f32, tag="iota_f")
        nc.scalar.copy(iota_f[:], iotl[:])
        niol = pp.tile([P, 1], i32, tag="niol")
        nc.gpsimd.iota(niol[:], pattern=[[1, 1]], base=P, channel_multiplier=-1)
        nio = pp.tile([P, 1], f32, tag="nio")
        nc.scalar.copy(nio[:], niol[:])

        # ---- per-camera geometry constants ----
        cam_consts = []
        for cam in range(NCAM):
            kc = pp.tile([3, 3], f32, tag=f"kc_{cam}")
            ec = pp.tile([4, 4], f32, tag=f"ec_{cam}")
            tmc = pp.tile([4, 4], f32, tag=f"tc_{cam}")
            nc.sync.dma_start(kc[:], kmat[cam])
            nc.sync.dma_start(ec[:], emat[cam])
            nc.sync.dma_start(tmc[:], tmat_t[cam])
            m4p = psc.tile([4, 4], f32, tag="smallp")
            nc.tensor.matmul(m4p[:], lhsT=tmc[:], rhs=ec[:],
                             start=True, stop=True)
            m4 = pp.tile([4, 4], f32, tag=f"m4_{cam}")
            nc.vector.tensor_copy(m4[:], m4p[:])
            m4tp = psc.tile([4, 4], f32, tag="smallp")
            nc.tensor.transpose(out=m4tp[:], in_=m4[:], identity=ident[:4, :4])
            m4t = pp.tile([4, 4], f32, tag=f"m4t_{cam}")
            nc.vector.tensor_copy(m4t[:], m4tp[:])
            n3p = psc.tile([3, 3], f32, tag="smallp")
            nc.tensor.matmul(n3p[:], lhsT=m4t[:3, :3], rhs=kc[:],
                             start=True, stop=True)
            n3 = pp.tile([3, 3], f32, tag=f"n3_{cam}")
            nc.vector.tensor_copy(n3[:], n3p[:])
            pk = pp.tile([1, 12], f32, tag=f"pk_{cam}")
            for i in range(3):
                nc.gpsimd.dma_start(pk[:, 3 * i:3 * i + 3], n3[i:i + 1, :])
                nc.gpsimd.dma_start(pk[:, 9 + i:10 + i], m4[i:i + 1, 3:4])
            bc = pp.tile([P, 12], f32, tag=f"bc_{cam}")
            nc.gpsimd.partition_broadcast(bc[:], pk[:])
            cam_consts.append(bc)

        # ---- load depths + features ----
        bulk_cm = tc.tile_pool(name="bulk", bufs=1)
        bp = bulk_cm.__enter__()
        dtile = bp.tile([P, F], f32, tag="dtile")
        for cam in range(NCAM):
            cs = slice(cam * FCAM, (cam + 1) * FCAM)
            nc.sync.dma_start(
                dtile[:, cs],
                dep[cam * HW:(cam + 1) * HW].rearrange("(p f) -> p f", p=P))
        ptile = bp.tile([P, F * CG], bf16, tag="ptile")
        for cam in range(NCAM):
            nc.sync.dma_start(
                ptile[:, cam * FCAM * CG:(cam + 1) * FCAM * CG]
                .rearrange("p (f c) -> p f c", c=CG),
                feat[cam * HW:(cam + 1) * HW, :]
                .rearrange("(f p) c -> p f c", p=P))

        linfs = [bp.tile([P, FCAM], f32, tag=f"linf{n}", name=f"linf{n}")
                 for n in range(NCAM)]
        linms = [bp.tile([P, FCAM], f32, tag=f"linm{n}", name=f"linm{n}")
                 for n in range(NCAM)]

        # ---- per-camera lift interleaved with dedup + CCE-add scatter ----
        with tc.tile_pool(name="lift", bufs=2) as lp, \
             tc.tile_pool(name="work", bufs=3) as wp, \
             tc.tile_pool(name="psw", bufs=3, space="PSUM") as psw:
          for cam in range(NCAM):
            _lift_cam(tc, lp, dtile, cam_consts, linfs[cam][:], cam)
            nc.vector.tensor_scalar(out=linms[cam][:], in0=linfs[cam][:],
                                    scalar1=-float(SENT), scalar2=None,
                                    op0=AL.add)
            for fc in range(min(FCAM, max(0, NF - cam * FCAM))):
                f = cam * FCAM + fc
                col = linfs[cam][:, fc:fc + 1]
                rowv = psw.tile([P, P], f32, tag="rowv")
                nc.tensor.transpose(out=rowv[:],
                                    in_=col.to_broadcast([P, P]),
                                    identity=ident[:])
                eq_bf = wp.tile([P, P], bf16, tag="eq_bf")
                nc.vector.tensor_tensor(out=eq_bf[:],
                                        in0=col.to_broadcast([P, P]),
                                        in1=rowv[:], op=AL.is_equal)
                # val = eq*128 - t' ; m2 = max val = 128 - (first equal idx)
                val = wp.tile([P, P], bf16, tag="val")
                nc.vector.scalar_tensor_tensor(out=val[:], in0=eq_bf[:],
                                               scalar=float(P), op0=AL.mult,
                                               in1=iota_f[:], op1=AL.subtract)
                m2 = wp.tile([P, 1], bf16, tag="m2")
                nc.vector.tensor_reduce(out=m2[:], in_=val[:],
                                        axis=mybir.AxisListType.X, op=AL.max)
                first = wp.tile([P, 1], f32, tag="first")
                nc.vector.tensor_tensor(out=first[:], in0=m2[:], in1=nio[:],
                                        op=AL.is_equal)
                t1 = wp.tile([P, 1], f32, tag="t1")
                nc.vector.tensor_tensor(out=t1[:], in0=first[:],
                                        in1=linms[cam][:, fc:fc + 1],
                                        op=AL.mult)
                t2 = wp.tile([P, 1], f32, tag="t2")
                nc.vector.tensor_scalar(out=t2[:], in0=t1[:],
                                        scalar1=float(SENT), scalar2=None,
                                        op0=AL.add)
                di = wp.tile([P, 1], i32, tag="di")
                nc.vector.tensor_scalar(out=di[:], in0=t2[:],
                                        scalar1=float(NROWS), scalar2=None,
                                        op0=AL.min)
                tot = psw.tile([P, CG], f32, tag="tot")
                nc.tensor.matmul(tot[:], lhsT=eq_bf[:],
                                 rhs=ptile[:, f * CG:(f + 1) * CG],
                                 start=True, stop=True)
                totb = wp.tile([P, CG], bf16, tag="totb")
                nc.vector.tensor_copy(totb[:], tot[:])
                nc.gpsimd.indirect_dma_start(
                    out=grids[f % R],
                    out_offset=bass.IndirectOffsetOnAxis(ap=di[:], axis=0),
                    in_=totb[:],
                    in_offset=None,
                    bounds_check=NROWS,
                    oob_is_err=False,
                    compute_op=AL.add)

        # ---- merge replicas + z-max + store ----
        bulk_cm.__exit__(None, None, None)
        SL = X * Y
        HF = SL // P // 2  # half-slice free length (256)
        with tc.tile_pool(name="merge", bufs=1) as mp, \
             tc.tile_pool(name="tap", bufs=2 * R) as tp_:
            if NOMERGE:
                zt = mp.tile([P, HF, CG], f32, tag="zt")
                nc.vector.memset(zt[:], 0.0)
                for h in range(2):
                    orows = slice(h * (SL // 2), (h + 1) * (SL // 2))
                    nc.sync.dma_start(
                        out[orows, :].rearrange("(p f) c -> p f c", p=P),
                        zt[:])
                return
            for h in range(2):
                acc = mp.tile([P, HF, CG], f32, tag=f"acc{h}")
                zacc = mp.tile([P, HF, CG], f32, tag=f"zacc{h}")
                for z in range(Z):
                    tas = []
                    for r in range(R):
                        rows = slice(z * SL + h * (SL // 2),
                                     z * SL + (h + 1) * (SL // 2))
                        ta = tp_.tile([P, HF, CG], bf16, tag="ta")
                        nc.sync.dma_start(
                            ta[:],
                            grids[r][rows, :].rearrange("(p f) c -> p f c",
                                                        p=P))
                        tas.append(ta)
                    if R == 1:
                        zac = tas[0]
                    else:
                        zac = zacc
                        nc.vector.tensor_tensor(out=zacc[:], in0=tas[0][:],
                                                in1=tas[1][:], op=AL.add)
                        for r in range(2, R):
                            nc.vector.tensor_tensor(out=zacc[:], in0=zacc[:],
                                                    in1=tas[r][:], op=AL.add)
                    if z == 0:
                        nc.vector.tensor_copy(acc[:], zac[:])
                    else:
                        nc.vector.tensor_tensor(out=acc[:], in0=acc[:],
                                                in1=zac[:], op=AL.max)
                orows = slice(h * (SL // 2), (h + 1) * (SL // 2))
                nc.sync.dma_start(
                    out[orows, :].rearrange("(p f) c -> p f c", p=P), acc[:])


def kernel(feat_maps, depths, K, E, T):
    global _prog_cache
    feat_maps = np.asarray(feat_maps, np.float32)
    depths = np.asarray(depths, np.float32)
    K = np.asarray(K, np.float32)
    E = np.asarray(E, np.float32)
    T = np.asarray(T, np.float32)

    if _prog_cache is None:
        _prog_cache = _build_program()
    nc = _prog_cache

    in_maps = []
    for core in range(8):
        b, cg = core // 4, core % 4
        ch = slice(cg * CG, (cg + 1) * CG)
        frows = np.concatenate([
            np.ascontiguousarray(
                feat_maps[b * N + n, ch].transpose(1, 2, 0).reshape(HW, CG)
                .reshape(P, FCAM, CG).transpose(1, 0, 2).reshape(HW, CG))
            for n in range(N)], axis=0).astype(ml_dtypes.bfloat16)
        cams = slice(b * N, (b + 1) * N)
        im = {
            "feat": frows,
            "dep": np.ascontiguousarray(depths[b].reshape(NTOK)),
            "kmat": np.ascontiguousarray(K[cams]),
            "emat": np.ascontiguousarray(E[cams]),
            "tmat_t": np.ascontiguousarray(T[cams].transpose(0, 2, 1)),
        }
        for r in range(R):
            im[f"grid{r}"] = np.zeros((NROWS + 8, CG), ml_dtypes.bfloat16)
        in_maps.append(im)

    res = bass_utils.run_bass_kernel_spmd(nc, in_maps, core_ids=list(range(8)))
    outp = np.zeros((1, B * C, X, Y), np.float32)
    for core in range(8):
        b, cg = core // 4, core % 4
        o = np.asarray(res.results[core]["out"], np.float32).reshape(X, Y, CG)
        outp[0, b * C + cg * CG:b * C + (cg + 1) * CG] = \
            o.transpose(2, 0, 1)[:, ::-1, ::-1]
    return outp


# revision 9
# speedup vs baseline: 1.2182x; 1.2182x over previous
"""BEVLiftNet Trainium2 kernel, v2.

Sharding: 8 cores = 2 batches x 4 channel-groups (16 ch each).
Per core: lift all 4 cameras of its batch (depths -> voxel linear ids),
then scatter-add 16-channel bf16 feature rows into R replica DRAM grids
[Z*X*Y, 16] via single CCE-add indirect DMAs (the DMA engine does the
read-modify-write inline, so each chunk costs ONE SWDGE call instead of
the gather+add+scatter round trip).  Within-chunk duplicate rows are
pre-merged with a selection-matrix matmul; loser rows and invalid pixels
are clamped to an in-bounds trash row (never read back) so no DMA error
notifications fire.  Round-robin over R replica grids keeps chains independent so the
GpSimd SWDGE emission pipeline never stalls on DMA completion.  A final
pass sums the replicas, max-reduces over Z, and writes f32; the host
reassembles the [1, B*C, X, Y] output (flips + channel concat).
"""

import os
import sys

sys.path.insert(0, "/opt/trn_rl_repo")

import numpy as np
import ml_dtypes

import concourse.bacc as bacc
import concourse.bass as bass
import concourse.mybir as mybir
import concourse.tile as tile
from concourse import bass_utils
from concourse.masks import make_identity

B, N, C, H, W = 2, 4, 64, 128, 352
X, Y, Z = 256, 256, 8
CG = 16           # channels per core
NCAM = 4          # cameras per core (one batch)
HW = H * W        # 45056 pixels per camera
NTOK = NCAM * HW  # 180224 tokens per core
P = 128
FCAM = HW // P    # 352 chunk-columns per camera
F = NCAM * FCAM   # 1408 chunk-columns total
NROWS = Z * X * Y          # 524288 voxel rows per grid
SENT = 8_000_000           # OOB sentinel (> NROWS-1, f32-exact)
R = int(os.environ.get("KBEV_R", "3"))    # replica grids / chains
NF = int(os.environ.get("KBEV_NF", str(F)))   # chunks to scatter (debug)
NOMERGE = os.environ.get("KBEV_NOMERGE", "") == "1"
f32 = mybir.dt.float32
bf16 = mybir.dt.bfloat16
i32 = mybir.dt.int32
AL = mybir.AluOpType
ACT = mybir.ActivationFunctionType

_prog_cache = None


def _build_program():
    nc = bacc.Bacc("TRN2", target_bir_lowering=False, debug=False)
    feat = nc.dram_tensor("feat", [NTOK, CG], bf16, kind="ExternalInput")
    dep = nc.dram_tensor("dep", [NTOK], f32, kind="ExternalInput")
    kmat = nc.dram_tensor("kmat", [NCAM, 3, 3], f32, kind="ExternalInput")
    emat = nc.dram_tensor("emat", [NCAM, 4, 4], f32, kind="ExternalInput")
    tmat_t = nc.dram_tensor("tmat_t", [NCAM, 4, 4], f32, kind="ExternalInput")
    out = nc.dram_tensor("out", [X * Y, CG], f32, kind="ExternalOutput")
    grids = [nc.dram_tensor(f"grid{r}", [NROWS + 8, CG], bf16,
                            kind="ExternalInput") for r in range(R)]
    with tile.TileContext(nc) as tc:
        _emit(tc, feat.ap(), dep.ap(), kmat.ap(), emat.ap(), tmat_t.ap(),
              out.ap(), [g.ap() for g in grids])
    nc.compile()
    return nc


def _floor(nc, wp, out_t, in_t, n):
    """out = floor(in), robust to any f32->i32 rounding mode."""
    ii = wp.tile([P, n], i32, tag="fl_i")
    nc.vector.tensor_copy(ii[:], in_t)
    ff = wp.tile([P, n], f32, tag="fl_f")
    nc.vector.tensor_copy(ff[:], ii[:])
    gt = wp.tile([P, n], f32, tag="fl_g")
    nc.vector.tensor_tensor(out=gt[:], in0=ff[:], in1=in_t, op=AL.is_gt)
    nc.vector.tensor_tensor(out=out_t, in0=ff[:], in1=gt[:], op=AL.subtract)


def _lift_cam(tc, wp, dtile, cam_consts, lf, cam):
    """Compute one camera's voxel linear index (f32), invalid -> SENT."""
    nc = tc.nc
    if True:
        if True:
            cs = slice(cam * FCAM, (cam + 1) * FCAM)
            bc = cam_consts[cam]
            d = dtile[:, cs]
            idl = wp.tile([P, FCAM], i32, tag="idl")
            nc.gpsimd.iota(idl[:], pattern=[[1, FCAM]], base=0,
                           channel_multiplier=FCAM)
            idf = wp.tile([P, FCAM], f32, tag="idf")
            nc.scalar.copy(idf[:], idl[:])
            yf = wp.tile([P, FCAM], f32, tag="yf")
            xf = wp.tile([P, FCAM], f32, tag="xf")
            tmp = wp.tile([P, FCAM], f32, tag="t0")
            nc.scalar.activation(tmp[:], idf[:], ACT.Copy, bias=0.5 / W,
                                 scale=1.0 / W)
            _floor(nc, wp, yf[:], tmp[:], FCAM)
            nc.vector.scalar_tensor_tensor(out=xf[:], in0=yf[:],
                                           scalar=-float(W), in1=idf[:],
                                           op0=AL.mult, op1=AL.add)
            gs = []
            for i in range(3):
                a = wp.tile([P, FCAM], f32, tag="a_i")
                nc.vector.tensor_scalar(out=a[:], in0=xf[:],
                                        scalar1=bc[:, 3 * i:3 * i + 1],
                                        scalar2=None, op0=AL.mult)
                nc.vector.scalar_tensor_tensor(
                    out=a[:], in0=yf[:], scalar=bc[:, 3 * i + 1:3 * i + 2],
                    in1=a[:], op0=AL.mult, op1=AL.add)
                nc.vector.tensor_scalar(out=a[:], in0=a[:],
                                        scalar1=bc[:, 3 * i + 2:3 * i + 3],
                                        scalar2=None, op0=AL.add)
                e = wp.tile([P, FCAM], f32, tag="e_i")
                nc.vector.tensor_tensor(out=e[:], in0=a[:], in1=d, op=AL.mult)
                nc.vector.tensor_scalar(out=e[:], in0=e[:],
                                        scalar1=bc[:, 9 + i:10 + i],
                                        scalar2=None, op0=AL.add)
                mid = (X / 2.0, Y / 2.0, Z / 2.0)[i]
                g = wp.tile([P, FCAM], f32, tag=f"g_{i}")
                nc.scalar.activation(g[:], e[:], ACT.Copy, bias=mid, scale=2.0)
                gs.append(g)
            gx, gy, gz = gs
            v = wp.tile([P, FCAM], f32, tag="v")
            nc.vector.tensor_scalar(out=v[:], in0=gx[:], scalar1=-1.0,
                                    scalar2=None, op0=AL.is_gt)
            nc.vector.scalar_tensor_tensor(out=v[:], in0=gx[:],
                                           scalar=float(X), in1=v[:],
                                           op0=AL.is_lt, op1=AL.logical_and)
            for gg, bound in ((gy, float(Y)), (gz, float(Z))):
                v2 = wp.tile([P, FCAM], f32, tag="v2")
                nc.vector.tensor_scalar(out=v2[:], in0=gg[:], scalar1=-1.0,
                                        scalar2=None, op0=AL.is_gt)
                nc.vector.scalar_tensor_tensor(out=v2[:], in0=gg[:],
                                               scalar=bound, in1=v2[:],
                                               op0=AL.is_lt,
                                               op1=AL.logical_and)
                nc.vector.tensor_tensor(out=v[:], in0=v[:], in1=v2[:],
                                        op=AL.logical_and)
            fx = wp.tile([P, FCAM], f32, tag="fx")
            fy = wp.tile([P, FCAM], f32, tag="fy")
            fz = wp.tile([P, FCAM], f32, tag="fz")
            _floor(nc, wp, fx[:], gx[:], FCAM)
            _floor(nc, wp, fy[:], gy[:], FCAM)
            _floor(nc, wp, fz[:], gz[:], FCAM)
            for ft in (fx, fy, fz):
                nc.vector.tensor_scalar(out=ft[:], in0=ft[:], scalar1=0.0,
                                        scalar2=255.0, op0=AL.max, op1=AL.min)
            nc.vector.scalar_tensor_tensor(out=lf, in0=fz[:], scalar=float(X),
                                           in1=fx[:], op0=AL.mult, op1=AL.add)
            nc.vector.scalar_tensor_tensor(out=lf, in0=lf, scalar=float(Y),
                                           in1=fy[:], op0=AL.mult, op1=AL.add)
            # blend invalid -> SENT :  lin = SENT + v*(lin-SENT)
            nc.vector.tensor_scalar(out=lf, in0=lf, scalar1=-float(SENT),
                                    scalar2=None, op0=AL.add)
            nc.vector.tensor_tensor(out=lf, in0=lf, in1=v[:], op=AL.mult)
            nc.vector.tensor_scalar(out=lf, in0=lf, scalar1=float(SENT),
                                    scalar2=None, op0=AL.add)


def _emit(tc, feat, dep, kmat, emat, tmat_t, out, grids):
    nc = tc.nc

    with tc.tile_pool(name="persist", bufs=1) as pp, \
         tc.tile_pool(name="psc", bufs=2, space="PSUM") as psc:

        ident = pp.tile([P, P], f32, tag="ident")
        make_identity(nc, ident[:])
        iotl = pp.tile([P, P], i32, tag="iotl")
        nc.gpsimd.iota(iotl[:], pattern=[[1, P]], base=0, channel_multiplier=0)
        iota_f = pp.tile([P, P], f32, tag="iota_f")
        nc.scalar.copy(iota_f[:], iotl[:])
        niol = pp.tile([P, 1], i32, tag="niol")
        nc.gpsimd.iota(niol[:], pattern=[[1, 1]], base=P, channel_multiplier=-1)
        nio = pp.tile([P, 1], f32, tag="nio")
        nc.scalar.copy(nio[:], niol[:])

        # ---- per-camera geometry constants ----
        cam_consts = []
        for cam in range(NCAM):
            kc = pp.tile([3, 3], f32, tag=f"kc_{cam}")
            ec = pp.tile([4, 4], f32, tag=f"ec_{cam}")
            tmc = pp.tile([4, 4], f32, tag=f"tc_{cam}")
            nc.sync.dma_start(kc[:], kmat[cam])
            nc.sync.dma_start(ec[:], emat[cam])
            nc.sync.dma_start(tmc[:], tmat_t[cam])
            m4p = psc.tile([4, 4], f32, tag="smallp")
            nc.tensor.matmul(m4p[:], lhsT=tmc[:], rhs=ec[:],
                             start=True, stop=True)
            m4 = pp.tile([4, 4], f32, tag=f"m4_{cam}")
            nc.vector.tensor_copy(m4[:], m4p[:])
            m4tp = psc.tile([4, 4], f32, tag="smallp")
            nc.tensor.transpose(out=m4tp[:], in_=m4[:], identity=ident[:4, :4])
            m4t = pp.tile([4, 4], f32, tag=f"m4t_{cam}")
            nc.vector.tensor_copy(m4t[:], m4tp[:])
            n3p = psc.tile([3, 3], f32, tag="smallp")
            nc.tensor.matmul(n3p[:], lhsT=m4t[:3, :3], rhs=kc[:],
                             start=True, stop=True)
            n3 = pp.tile([3, 3], f32, tag=f"n3_{cam}")
            nc.vector.tensor_copy(n3[:], n3p[:])
            pk = pp.tile([1, 12], f32, tag=f"pk_{cam}")
            for i in range(3):
                nc.gpsimd.dma_start(pk[:, 3 * i:3 * i + 3], n3[i:i + 1, :])
                nc.gpsimd.dma_start(pk[:, 9 + i:10 + i], m4[i:i + 1, 3:4])
            bc = pp.tile([P, 12], f32, tag=f"bc_{cam}")
            nc.gpsimd.partition_broadcast(bc[:], pk[:])
            cam_consts.append(bc)

        # ---- load depths + features ----
        bulk_cm = tc.tile_pool(name="bulk", bufs=1)
        bp = bulk_cm.__enter__()
        dtile = bp.tile([P, F], f32, tag="dtile")
        for cam in range(NCAM):
            cs = slice(cam * FCAM, (cam + 1) * FCAM)
            nc.sync.dma_start(
                dtile[:, cs],
                dep[cam * HW:(cam + 1) * HW].rearrange("(p f) -> p f", p=P))
        ptile = bp.tile([P, F * CG], bf16, tag="ptile")
        for cam in range(NCAM):
            nc.sync.dma_start(
                ptile[:, cam * FCAM * CG:(cam + 1) * FCAM * CG]
                .rearrange("p (f c) -> p f c", c=CG),
                feat[cam * HW:(cam + 1) * HW, :]
                .rearrange("(f p) c -> p f c", p=P))

        linfs = [bp.tile([P, FCAM], f32, tag=f"linf{n}", name=f"linf{n}")
                 for n in range(NCAM)]
        linms = [bp.tile([P, FCAM], f32, tag=f"linm{n}", name=f"linm{n}")
                 for n in range(NCAM)]

        # ---- per-camera lift interleaved with dedup + CCE-add scatter ----
        with tc.tile_pool(name="lift", bufs=2) as lp, \
             tc.tile_pool(name="work", bufs=3) as wp, \
             tc.tile_pool(name="psw", bufs=3, space="PSUM") as psw:
          for cam in range(NCAM):
            _lift_cam(tc, lp, dtile, cam_consts, linfs[cam][:], cam)
            nc.vector.tensor_scalar(out=linms[cam][:], in0=linfs[cam][:],
                                    scalar1=-float(SENT), scalar2=None,
                                    op0=AL.add)
            for fc in range(min(FCAM, max(0, NF - cam * FCAM))):
                f = cam * FCAM + fc
                col = linfs[cam][:, fc:fc + 1]
                rowv = psw.tile([P, P], f32, tag="rowv")
                nc.tensor.transpose(out=rowv[:],
                                    in_=col.to_broadcast([P, P]),
                                    identity=ident[:])
                eq_bf = wp.tile([P, P], bf16, tag="eq_bf")
                nc.vector.tensor_tensor(out=eq_bf[:],
                                        in0=col.to_broadcast([P, P]),
                                        in1=rowv[:], op=AL.is_equal)
                # val = eq*128 - t' ; m2 = max val = 128 - (first equal idx)
                val = wp.tile([P, P], bf16, tag="val")
                nc.vector.scalar_tensor_tensor(out=val[:], in0=eq_bf[:],
                                               scalar=float(P), op0=AL.mult,
                                               in1=iota_f[:], op1=AL.subtract)
                m2 = wp.tile([P, 1], bf16, tag="m2")
                nc.vector.tensor_reduce(out=m2[:], in_=val[:],
                                        axis=mybir.AxisListType.X, op=AL.max)
                first = wp.tile([P, 1], f32, tag="first")
                nc.vector.tensor_tensor(out=first[:], in0=m2[:], in1=nio[:],
                                        op=AL.is_equal)
                t1 = wp.tile([P, 1], f32, tag="t1")
                nc.vector.tensor_tensor(out=t1[:], in0=first[:],
                                        in1=linms[cam][:, fc:fc + 1],
                                        op=AL.mult)
                t2 = wp.tile([P, 1], f32, tag="t2")
                nc.vector.tensor_scalar(out=t2[:], in0=t1[:],
                                        scalar1=float(SENT), scalar2=None,
                                        op0=AL.add)
                di = wp.tile([P, 1], i32, tag="di")
                nc.vector.tensor_scalar(out=di[:], in0=t2[:],
                                        scalar1=float(NROWS), scalar2=None,
                                        op0=AL.min)
                tot = psw.tile([P, CG], f32, tag="tot")
                nc.tensor.matmul(tot[:], lhsT=eq_bf[:],
                                 rhs=ptile[:, f * CG:(f + 1) * CG],
                                 start=True, stop=True)
                totb = wp.tile([P, CG], bf16, tag="totb")
                nc.vector.tensor_copy(totb[:], tot[:])
                nc.gpsimd.indirect_dma_start(
                    out=grids[f % R],
                    out_offset=bass.IndirectOffsetOnAxis(ap=di[:], axis=0),
                    in_=totb[:],
                    in_offset=None,
                    bounds_check=NROWS,
                    oob_is_err=False,
                    compute_op=AL.add)

        # ---- merge replicas + z-max + store ----
        bulk_cm.__exit__(None, None, None)
        SL = X * Y
        HF = SL // P // 2  # half-slice free length (256)
        with tc.tile_pool(name="merge", bufs=1) as mp, \
             tc.tile_pool(name="tap", bufs=2 * R) as tp_:
            if NOMERGE:
                zt = mp.tile([P, HF, CG], f32, tag="zt")
                nc.vector.memset(zt[:], 0.0)
                for h in range(2):
                    orows = slice(h * (SL // 2), (h + 1) * (SL // 2))
                    nc.sync.dma_start(
                        out[orows, :].rearrange("(p f) c -> p f c", p=P),
                        zt[:])
                return
            for h in range(2):
                acc = mp.tile([P, HF, CG], f32, tag=f"acc{h}")
                # pair-add in bf16 engages the DVE 16-bit fast path; the
                # final accumulate/max stay f32 (one extra rounding only)
                zacc = mp.tile([P, HF, CG], bf16, tag=f"zacc{h}")
                zsum = mp.tile([P, HF, CG], f32, tag=f"zsum{h}")
                for z in range(Z):
                    tas = []
                    for r in range(R):
                        rows = slice(z * SL + h * (SL // 2),
                                     z * SL + (h + 1) * (SL // 2))
                        ta = tp_.tile([P, HF, CG], bf16, tag="ta")
                        nc.sync.dma_start(
                            ta[:],
                            grids[r][rows, :].rearrange("(p f) c -> p f c",
                                                        p=P))
                        tas.append(ta)
                    if R == 1:
                        zac = tas[0]
                    elif R == 2:
                        zac = zsum
                        nc.vector.tensor_tensor(out=zsum[:], in0=tas[0][:],
                                                in1=tas[1][:], op=AL.add)
                    else:
                        zac = zsum
                        nc.vector.tensor_tensor(out=zacc[:], in0=tas[0][:],
                                                in1=tas[1][:], op=AL.add)
                        for r in range(2, R - 1):
                            nc.vector.tensor_tensor(out=zacc[:], in0=zacc[:],
                                                    in1=tas[r][:], op=AL.add)
                        nc.vector.tensor_tensor(out=zsum[:], in0=zacc[:],
                                                in1=tas[R - 1][:], op=AL.add)
                    if z == 0:
                        nc.vector.tensor_copy(acc[:], zac[:])
                    else:
                        nc.vector.tensor_tensor(out=acc[:], in0=acc[:],
                                                in1=zac[:], op=AL.max)
                orows = slice(h * (SL // 2), (h + 1) * (SL // 2))
                nc.sync.dma_start(
                    out[orows, :].rearrange("(p f) c -> p f c", p=P), acc[:])


def kernel(feat_maps, depths, K, E, T):
    global _prog_cache
    feat_maps = np.asarray(feat_maps, np.float32)
    depths = np.asarray(depths, np.float32)
    K = np.asarray(K, np.float32)
    E = np.asarray(E, np.float32)
    T = np.asarray(T, np.float32)

    if _prog_cache is None:
        _prog_cache = _build_program()
    nc = _prog_cache

    in_maps = []
    for core in range(8):
        b, cg = core // 4, core % 4
        ch = slice(cg * CG, (cg + 1) * CG)
        frows = np.concatenate([
            np.ascontiguousarray(
                feat_maps[b * N + n, ch].transpose(1, 2, 0).reshape(HW, CG)
                .reshape(P, FCAM, CG).transpose(1, 0, 2).reshape(HW, CG))
            for n in range(N)], axis=0).astype(ml_dtypes.bfloat16)
        cams = slice(b * N, (b + 1) * N)
        im = {
            "feat": frows,
            "dep": np.ascontiguousarray(depths[b].reshape(NTOK)),
            "kmat": np.ascontiguousarray(K[cams]),
            "emat": np.ascontiguousarray(E[cams]),
            "tmat_t": np.ascontiguousarray(T[cams].transpose(0, 2, 1)),
        }
        for r in range(R):
            im[f"grid{r}"] = np.zeros((NROWS + 8, CG), ml_dtypes.bfloat16)
        in_maps.append(im)

    res = bass_utils.run_bass_kernel_spmd(nc, in_maps, core_ids=list(range(8)))
    outp = np.zeros((1, B * C, X, Y), np.float32)
    for core in range(8):
        b, cg = core // 4, core % 4
        o = np.asarray(res.results[core]["out"], np.float32).reshape(X, Y, CG)
        outp[0, b * C + cg * CG:b * C + (cg + 1) * CG] = \
            o.transpose(2, 0, 1)[:, ::-1, ::-1]
    return outp
